# revision 18
# baseline (speedup 1.0000x reference)
"""Trainium2 Bass kernel for the L2D job-shop GNN encoder.

Problem: B=64 batches, J=50 jobs x M=20 machine-ops = N=1000 nodes, E=128,
FF=512, L=3 GNN layers.  Data-parallel over 8 NeuronCores (8 batches each).

Key algebraic restructure vs. the reference:
  - adj_prec aggregation == shift h by one token (zero at job boundaries)
  - adj_mach aggregation == P @ (P^T h) - h  with P the [N,20] one-hot of
    machine ids  ->  two tiny matmuls instead of a [1000x1000] dense matmul.
    (P S) @ Wm == P @ (S @ Wm), and the "- h @ Wm" term folds into
    (Ws - Wm) as the self-weight.
Everything runs feature-major (H = h^T, [E=128 partitions, N tokens free]);
token-contractions (machine segment-sum, final gather) use 8 TensorE
transposes per batch/layer into token-major [125,128] chunks.

All large matmuls run in float32r (single-pass replicated fp32, 4x the
fp32 rate for moving dim >= 256); producers of matmul operands write
float32r so the BIR verifier's rounding requirement is met.
"""

import numpy as np

B, J, M = 64, 50, 20
N = J * M            # 1000
E, FF, L = 128, 512, 3
NCORES = 8
BPC = B // NCORES    # 8 batches per core
CH = 8               # token chunks per batch
CP = N // CH         # 125 tokens per chunk
HALF = N // 2        # 500 (psum bank = 512 fp32)
NP = 1024            # padded token count (DMA-transpose needs 32-aligned tiles)
SCALE = 2048.0       # h is computed as h/SCALE on device (fp16 range safety)

_CACHE = {}


def _build_nc():
    import concourse.bass as bass  # noqa: F401
    import concourse.mybir as mybir
    import concourse.tile as tile
    from concourse import bacc
    from concourse.masks import make_identity

    dt = mybir.dt
    f32 = dt.float32
    f32r = dt.float32r  # noqa: F841
    f16 = dt.float16
    i32 = dt.int32
    u8 = dt.uint8
    AF = mybir.ActivationFunctionType
    OP = mybir.AluOpType

    nc = bacc.Bacc(
        "TRN2",
        target_bir_lowering=False,
        debug=False,
        enable_asserts=False,
        num_devices=NCORES,
    )

    proc = nc.dram_tensor("proc_time", [BPC, N], f32, kind="ExternalInput")
    mo = nc.dram_tensor("machine_order", [BPC, N], i32, kind="ExternalInput")
    nxt = nc.dram_tensor("next_op_idx", [BPC, J], i32, kind="ExternalInput")
    fin = nc.dram_tensor("finished_jobs", [BPC, J], u8, kind="ExternalInput")
    w_init = nc.dram_tensor("W_init", [1, E], f32, kind="ExternalInput")
    b_init = nc.dram_tensor("b_init", [E], f32, kind="ExternalInput")
    Ws_d, Wp_d, Wm_d, b_d, W1_d, b1_d, W2_d, b2_d = [], [], [], [], [], [], [], []
    for l in range(L):
        Ws_d.append(nc.dram_tensor(f"Ws{l}", [E, E], f32, kind="ExternalInput"))
        Wp_d.append(nc.dram_tensor(f"Wp{l}", [E, E], f32, kind="ExternalInput"))
        Wm_d.append(nc.dram_tensor(f"Wm{l}", [E, E], f32, kind="ExternalInput"))
        b_d.append(nc.dram_tensor(f"b{l}", [E], f32, kind="ExternalInput"))
        W1_d.append(nc.dram_tensor(f"W1{l}", [E, FF], f32, kind="ExternalInput"))
        b1_d.append(nc.dram_tensor(f"b1{l}", [FF], f32, kind="ExternalInput"))
        W2_d.append(nc.dram_tensor(f"W2{l}", [FF, E], f32, kind="ExternalInput"))
        b2_d.append(nc.dram_tensor(f"b2{l}", [E], f32, kind="ExternalInput"))
    h_out = nc.dram_tensor("h_out", [BPC, N, E], f16, kind="ExternalOutput")
    je_out = nc.dram_tensor("je_out", [BPC, J, E], f32, kind="ExternalOutput")

    with tile.TileContext(nc) as tc:
        with (
            tc.tile_pool(name="const", bufs=1) as const,
            tc.tile_pool(name="wpool", bufs=1) as wpool,
            tc.tile_pool(name="inp", bufs=4) as inp,
            tc.tile_pool(name="dramp", bufs=3, space="DRAM") as dramp,
            tc.tile_pool(name="persist", bufs=BPC + 1) as persist,
            tc.tile_pool(name="hpool", bufs=BPC + 4) as hpool,
            tc.tile_pool(name="apool", bufs=4) as apool,
            tc.tile_pool(name="htokp", bufs=4) as htokp,
            tc.tile_pool(name="msgp", bufs=4) as msgp,
            tc.tile_pool(name="tpool", bufs=5) as tpool,
            tc.tile_pool(name="smallsb", bufs=5) as smallsb,
            tc.tile_pool(name="small_ps", bufs=2, space="PSUM") as small_ps,
            tc.tile_pool(name="mt_ps", bufs=3, space="PSUM") as mt_ps,
            tc.tile_pool(name="hn_ps", bufs=2, space="PSUM") as hn_ps,
        ):
            # ---------------- constants ----------------
            ident = const.tile([128, 128], f32)
            make_identity(nc, ident[:])
            ones = const.tile([1, 128], f32)
            nc.gpsimd.memset(ones[:], 1.0)
            iota20i = const.tile([20, 1], i32)
            nc.gpsimd.iota(iota20i[:], pattern=[[0, 1]], base=0, channel_multiplier=1)
            iota20f = const.tile([20, 1], f32)
            nc.vector.tensor_copy(iota20f[:], iota20i[:])
            # token id of (partition p, chunk c) after DMA transpose = c*128 + p
            tokidi = const.tile([E, CH], i32)
            nc.gpsimd.iota(tokidi[:], pattern=[[E, CH]], base=0,
                           channel_multiplier=1)
            tokidf = const.tile([E, CH], f32)
            nc.vector.tensor_copy(tokidf[:], tokidi[:])
            iotami = const.tile([E, CH * M], i32)
            nc.gpsimd.iota(
                iotami[:], pattern=[[0, CH], [1, M]], base=0, channel_multiplier=0
            )
            iotamf = const.tile([E, CH * M], f32)
            nc.vector.tensor_copy(iotamf[:], iotami[:])
            iotaji = const.tile([1, J], i32)
            nc.gpsimd.iota(iotaji[:], pattern=[[M, J]], base=0, channel_multiplier=0)
            iotajf = const.tile([1, J], f32)
            nc.vector.tensor_copy(iotajf[:], iotaji[:])
            zero_f = const.tile([E, 1], f32)
            nc.gpsimd.memset(zero_f[:], 0.0)
            zero_h = const.tile([E, 1], f16)
            nc.vector.tensor_copy(zero_h[:], zero_f[:])
            ident_h = const.tile([128, 128], f16)
            nc.vector.tensor_copy(ident_h[:], ident[:])

            # ---------------- weights ----------------
            winit_sb = wpool.tile([E, 1], f32)
            nc.sync.dma_start(winit_sb[:], w_init.rearrange("o p -> p o"))
            binit_sb = wpool.tile([E, 1], f32)
            nc.sync.dma_start(binit_sb[:], b_init.rearrange("(p o) -> p o", o=1))
            wsm_sb, wp_sb, wm_sb, b_sb, w1_sb, b1_sb, w2_sb, b2_sb = (
                [], [], [], [], [], [], [], []
            )
            for l in range(L):
                ws_t = wpool.tile([E, E], f32, name=f"ws{l}")
                nc.sync.dma_start(ws_t[:], Ws_d[l][:])
                wpf_t = wpool.tile([E, E], f32, name=f"wpf{l}")
                nc.sync.dma_start(wpf_t[:], Wp_d[l][:])
                wmf_t = wpool.tile([E, E], f32, name=f"wmf{l}")
                nc.sync.dma_start(wmf_t[:], Wm_d[l][:])
                # f32r (rounded) weight copies for the PE
                wsm_t = wpool.tile([E, E], f16, name=f"wsm{l}")
                nc.vector.tensor_sub(wsm_t[:], ws_t[:], wmf_t[:])
                wp_t = wpool.tile([E, E], f16, name=f"wp{l}")
                nc.vector.tensor_copy(wp_t[:], wpf_t[:])
                wm_t = wpool.tile([E, E], f16, name=f"wm{l}")
                nc.vector.tensor_copy(wm_t[:], wmf_t[:])
                b_t = wpool.tile([E, 1], f32, name=f"b{l}")
                nc.sync.dma_start(b_t[:], b_d[l].rearrange("(p o) -> p o", o=1))
                w1f_t = wpool.tile([E, FF], f32, name=f"w1f{l}")
                nc.sync.dma_start(w1f_t[:], W1_d[l][:])
                w1_t = wpool.tile([E, FF], f16, name=f"w1{l}")
                nc.vector.tensor_copy(w1_t[:], w1f_t[:])
                b1_t = wpool.tile([E, FF // E], f32, name=f"b1{l}")
                nc.sync.dma_start(b1_t[:], b1_d[l].rearrange("(f p) -> p f", p=E))
                w2f_t = wpool.tile([E, FF // E, E], f32, name=f"w2f{l}")
                nc.sync.dma_start(w2f_t[:], W2_d[l].rearrange("(f p) e -> p f e", p=E))
                w2_t = wpool.tile([E, FF // E, E], f16, name=f"w2{l}")
                nc.vector.tensor_copy(w2_t[:], w2f_t[:])
                b2_t = wpool.tile([E, 1], f32, name=f"b2{l}")
                nc.sync.dma_start(b2_t[:], b2_d[l].rearrange("(p o) -> p o", o=1))
                wsm_sb.append(wsm_t)
                wp_sb.append(wp_t)
                wm_sb.append(wm_t)
                b_sb.append(b_t)
                w1_sb.append(w1_t)
                b1_sb.append(b1_t)
                w2_sb.append(w2_t)
                b2_sb.append(b2_t)

            nf = FF // E  # 4

            # ---------- prep thunks (interleaved into early denses) ----------
            pts, p_alls, g_alls, h_curs = (
                [None] * BPC, [None] * BPC, [None] * BPC, [None] * BPC
            )

            def emit_prep(b):
                st = {}
                thunks = []

                def p_dma():
                    st["mo_bc"] = inp.tile([20, N], i32, name="mo_bc")
                    nc.sync.dma_start(st["mo_bc"][:], mo[b].partition_broadcast(20))
                    st["motok_i"] = inp.tile([E, CH], i32, name="motok_i")
                    nc.gpsimd.memset(st["motok_i"][:], -1)
                    nfull = N // E  # 7 full chunks of 128 tokens
                    nc.sync.dma_start(
                        st["motok_i"][:, 0:nfull],
                        mo[b][0:nfull * E].rearrange("(c p) -> p c", p=E),
                    )
                    nc.sync.dma_start(
                        st["motok_i"][0:N - nfull * E, nfull:nfull + 1],
                        mo[b][nfull * E:N].rearrange("(p o) -> p o", o=1),
                    )
                    st["nxt_i"] = inp.tile([1, J], i32, name="nxt_i")
                    nc.sync.dma_start(st["nxt_i"][:], nxt[b][None, :])
                    st["fin_u"] = inp.tile([1, J], u8, name="fin_u")
                    nc.sync.dma_start(st["fin_u"][:], fin[b][None, :])
                    st["dur_bc"] = inp.tile([E, N], f32, name="dur_bc")
                    nc.sync.dma_start(
                        st["dur_bc"][:], proc[b].partition_broadcast(E)
                    )

                def p_onehot():
                    motok_f = inp.tile([E, CH], f32, name="motok_f")
                    nc.vector.tensor_copy(motok_f[:], st["motok_i"][:])
                    p_all = persist.tile([E, CH, M], f16, name="p_all")
                    nc.vector.tensor_tensor(
                        p_all[:],
                        motok_f[:][:, :, None].broadcast_to([E, CH, M]),
                        iotamf[:].rearrange("p (c m) -> p c m", m=M),
                        op=OP.is_equal,
                    )
                    p_alls[b] = p_all

                def p_pt():
                    pt = persist.tile([20, N], f16, name="pt")
                    nc.vector.tensor_scalar(
                        pt[:], st["mo_bc"][:], iota20f[:, 0:1], None,
                        op0=OP.is_equal,
                    )
                    pts[b] = pt

                def p_h0():
                    h_cur = hpool.tile([E, NP], f16, name="h0", tag="h")
                    nc.scalar.activation(
                        h_cur[:, 0:N], st["dur_bc"][:], AF.Identity,
                        bias=binit_sb[:, 0:1], scale=winit_sb[:, 0:1],
                    )
                    nc.vector.tensor_copy(
                        h_cur[:, N:NP], zero_h[:, 0:1].broadcast_to([E, NP - N])
                    )
                    h_curs[b] = h_cur

                def p_flat():
                    nxt_f = inp.tile([1, J], f32, name="nxt_f")
                    nc.vector.tensor_copy(nxt_f[:], st["nxt_i"][:])
                    fin_f = inp.tile([1, J], f32, name="fin_f")
                    nc.vector.tensor_copy(fin_f[:], st["fin_u"][:])
                    flat_f = inp.tile([1, J], f32, name="flat_f")
                    nc.vector.tensor_scalar(
                        flat_f[:], nxt_f[:], -1.0, 19.0, op0=OP.mult, op1=OP.add
                    )
                    nc.vector.tensor_mul(flat_f[:], flat_f[:], fin_f[:])
                    nc.vector.tensor_add(flat_f[:], flat_f[:], nxt_f[:])
                    nc.vector.tensor_add(flat_f[:], flat_f[:], iotajf[:])
                    flat_d = dramp.tile([1, J], f32, name="flat_d")
                    nc.sync.dma_start(flat_d[:], flat_f[:])
                    st["flat_bc"] = inp.tile([E, J], f32, name="flat_bc")
                    nc.sync.dma_start(
                        st["flat_bc"][:], flat_d[0].partition_broadcast(E)
                    )

                def p_g():
                    g_all = persist.tile([E, CH, J], f16, name="g_all")
                    nc.vector.tensor_tensor(
                        g_all[:],
                        st["flat_bc"][:][:, None, :].broadcast_to([E, CH, J]),
                        tokidf[:][:, :, None].broadcast_to([E, CH, J]),
                        op=OP.is_equal,
                    )
                    g_alls[b] = g_all

                return [p_dma, p_onehot, p_pt, p_h0, p_flat, p_g]

            # ---------- software-pipelined layers ----------
            # "stall" = shift + transposes + segment-sum S + U for one (b, l):
            # sparse PE work that would starve the array (HAM re-throttle) if
            # emitted as a block, so it is interleaved into the previous
            # batch's dense msg/FFN matmul stream via thunks.
            def emit_stall(b, l):
                st = {}
                thunks = []

                def sh():
                    h_in = h_curs[b]
                    agg = apool.tile([E, N], f16, name="agg", tag="agg")
                    agg3 = agg[:].rearrange("p (j s) -> p j s", s=M)
                    h3 = h_in[:, 0:N].rearrange("p (j s) -> p j s", s=M)
                    nc.vector.tensor_copy(agg3[:, :, 0:M - 1], h3[:, :, 1:M])
                    nc.vector.tensor_copy(
                        agg3[:, :, M - 1], zero_h[:, 0:1].broadcast_to([E, J])
                    )
                    st["agg"] = agg
                    st["htok"] = htokp.tile([E, CH, E], f16, name="htok",
                                            tag="htok")

                thunks.append(sh)

                def tdma():
                    # token-major copy of h via DMA xbar transpose:
                    # htok[p, c, e] = h[token c*128+p, e]
                    nc.scalar.dma_start(
                        st["htok"][:], h_curs[b][:], transpose=True
                    )

                thunks.append(tdma)

                def mk_s(c0):
                    def smm():
                        if c0 == 0:
                            st["s_ps"] = small_ps.tile([E, M], f32, name="s_ps",
                                                       tag="sp")
                        for c in range(c0, c0 + 4):
                            nc.tensor.matmul(
                                st["s_ps"][:],
                                st["htok"][:, c, :],
                                p_alls[b][:, c, :],
                                start=(c == 0),
                                stop=(c == CH - 1),
                            )
                    return smm

                thunks.append(mk_s(0))
                thunks.append(mk_s(4))

                def ufin():
                    s_sb = smallsb.tile([E, M], f16, name="s_sb")
                    nc.vector.tensor_copy(s_sb[:], st["s_ps"][:])
                    u_ps = small_ps.tile([M, E], f32, name="u_ps", tag="sp")
                    nc.tensor.matmul(u_ps[:], s_sb[:], wm_sb[l][:])
                    u_sb = smallsb.tile([M, E], f16, name="u_sb")
                    nc.vector.tensor_copy(u_sb[:], u_ps[:])
                    st["u_sb"] = u_sb

                thunks.append(ufin)
                return thunks, st

            def emit_out(b):
                st = {}
                thunks = []

                def start():
                    st["htok_o"] = htokp.tile([E, CH, E], f16, name="htok_o",
                                              tag="htok")
                    nc.scalar.dma_start(
                        st["htok_o"][:], h_curs[b][:], transpose=True
                    )

                thunks.append(start)

                def mk_g(c0):
                    def gmm():
                        if c0 == 0:
                            st["je_ps"] = small_ps.tile([E, J], f32,
                                                        name="je_ps", tag="sp")
                        for c in range(c0, c0 + 4):
                            nc.tensor.matmul(
                                st["je_ps"][:],
                                st["htok_o"][:, c, :],
                                g_alls[b][:, c, :],
                                start=(c == 0),
                                stop=(c == CH - 1),
                            )
                    return gmm

                thunks.append(mk_g(0))
                thunks.append(mk_g(4))

                def fin():
                    je_sb = smallsb.tile([E, J], f32, name="je_sb")
                    nc.vector.tensor_scalar_mul(je_sb[:], st["je_ps"][:], SCALE)
                    jet_ps = small_ps.tile([J, E], f32, name="jet_ps", tag="sp")
                    nc.tensor.transpose(jet_ps[:], je_sb[:], ident[:])
                    jet_sb = smallsb.tile([J, E], f32, name="jet_sb")
                    nc.scalar.copy(jet_sb[:], jet_ps[:])
                    nc.sync.dma_start(je_out[b], jet_sb[:])
                    nfull = N // E
                    nc.sync.dma_start(
                        h_out[b][0:nfull * E].rearrange("(c p) e -> p c e", p=E),
                        st["htok_o"][:, 0:nfull, :],
                    )
                    nc.sync.dma_start(
                        h_out[b][nfull * E:N],
                        st["htok_o"][0:N - nfull * E, nfull, :],
                    )

                thunks.append(fin)
                return thunks

            def dense(b, l, st, pend):
                def fill(k):
                    for _ in range(k):
                        if pend:
                            pend.pop(0)()

                h_in = h_curs[b]
                agg = st["agg"]
                fill(2)
                msgs = []
                for hf in range(2):
                    sl = slice(hf * HALF, (hf + 1) * HALF)
                    m_ps = mt_ps.tile([E, HALF], f32, name="m_ps", tag="mt")
                    nc.tensor.matmul(
                        m_ps[:], wsm_sb[l][:], h_in[:, sl], start=True, stop=False
                    )
                    fill(1)
                    nc.tensor.matmul(
                        m_ps[:], wp_sb[l][:], agg[:, sl], start=False, stop=False
                    )
                    fill(1)
                    nc.tensor.matmul(
                        m_ps[:], st["u_sb"][:], pts[b][:, sl],
                        start=False, stop=True,
                    )
                    msg_t = msgp.tile([E, HALF], f16, name="msg_t")
                    nc.scalar.activation(
                        msg_t[:], m_ps[:], AF.Relu, bias=b_sb[l][:, 0:1]
                    )
                    msgs.append(msg_t)
                    fill(1)
                hn_pss = [
                    hn_ps.tile([E, HALF], f32, name="hn_ps0", tag="hn"),
                    hn_ps.tile([E, HALF], f32, name="hn_ps1", tag="hn"),
                ]
                for f in range(nf):
                    for hf in range(2):
                        tt_ps = mt_ps.tile([E, HALF], f32, name="tt_ps", tag="mt")
                        nc.tensor.matmul(
                            tt_ps[:], w1_sb[l][:, f * E:(f + 1) * E], msgs[hf][:]
                        )
                        t_sb = tpool.tile([E, HALF], f16, name="t_sb")
                        if f % 2 == 0:
                            nc.vector.tensor_scalar(
                                t_sb[:], tt_ps[:], b1_sb[l][:, f:f + 1], 0.0,
                                op0=OP.add, op1=OP.max,
                            )
                        else:
                            nc.scalar.activation(
                                t_sb[:], tt_ps[:], AF.Relu,
                                bias=b1_sb[l][:, f:f + 1],
                            )
                        nc.tensor.matmul(
                            hn_pss[hf][:], w2_sb[l][:, f, :], t_sb[:],
                            start=(f == 0), stop=(f == nf - 1),
                        )
                    fill(2)
                h_nxt = hpool.tile([E, NP], f16, name=f"h{l + 1}", tag="h")
                for hf in range(2):
                    sl = slice(hf * HALF, (hf + 1) * HALF)
                    nc.vector.scalar_tensor_tensor(
                        h_nxt[:, sl], hn_pss[hf][:], b2_sb[l][:, 0:1],
                        msgs[hf][:], op0=OP.add, op1=OP.add,
                    )
                nc.vector.tensor_copy(
                    h_nxt[:, N:NP], zero_h[:, 0:1].broadcast_to([E, NP - N])
                )
                h_curs[b] = h_nxt
                fill(len(pend))

            stall_sts = {}
            for bb in (0, 1, 2):
                for t in emit_prep(bb):
                    t()
            for bb in (0, 1):
                thunks, st = emit_stall(bb, 0)
                for t in thunks:
                    t()
                stall_sts[(bb, 0)] = st
            for l in range(L):
                for b in range(BPC):
                    st = stall_sts.pop((b, l))
                    pend = []
                    if l == 0 and b + 3 < BPC:
                        pend = emit_prep(b + 3)
                    tb = b + 2
                    if tb < BPC:
                        nxt_bl = (tb, l)
                    elif l + 1 < L:
                        nxt_bl = (tb - BPC, l + 1)
                    else:
                        nxt_bl = None
                    if nxt_bl is not None:
                        spend, pst = emit_stall(*nxt_bl)
                        pend = pend + spend
                        stall_sts[nxt_bl] = pst
                    if l == L - 1 and b >= 2:
                        pend = pend + emit_out(b - 2)
                    dense(b, l, st, pend)
            for bb in (BPC - 2, BPC - 1):
                for t in emit_out(bb):
                    t()

    nc.compile()
    return nc


def _get_nc():
    if "nc" not in _CACHE:
        _CACHE["nc"] = _build_nc()
    return _CACHE["nc"]


def make_in_maps(proc_time, machine_order, next_op_idx, finished_jobs, params):
    proc_time = np.asarray(proc_time, dtype=np.float32).reshape(B, N)
    machine_order = np.asarray(machine_order, dtype=np.int32).reshape(B, N)
    next_op_idx = np.asarray(next_op_idx, dtype=np.int32).reshape(B, J)
    finished_jobs = np.asarray(finished_jobs).astype(np.uint8).reshape(B, J)
    inv = np.float32(1.0 / SCALE)
    wmap = {
        "W_init": np.ascontiguousarray(np.asarray(params["W_init"], np.float32) * inv),
        "b_init": np.ascontiguousarray(np.asarray(params["b_init"], np.float32) * inv),
    }
    for l, lp in enumerate(params["layers"]):
        wmap[f"Ws{l}"] = np.ascontiguousarray(np.asarray(lp["Ws"], np.float32))
        wmap[f"Wp{l}"] = np.ascontiguousarray(np.asarray(lp["Wp"], np.float32))
        wmap[f"Wm{l}"] = np.ascontiguousarray(np.asarray(lp["Wm"], np.float32))
        wmap[f"b{l}"] = np.ascontiguousarray(np.asarray(lp["b"], np.float32) * inv)
        wmap[f"W1{l}"] = np.ascontiguousarray(np.asarray(lp["W1"], np.float32))
        wmap[f"b1{l}"] = np.ascontiguousarray(np.asarray(lp["b1"], np.float32) * inv)
        wmap[f"W2{l}"] = np.ascontiguousarray(np.asarray(lp["W2"], np.float32))
        wmap[f"b2{l}"] = np.ascontiguousarray(np.asarray(lp["b2"], np.float32) * inv)
    in_maps = []
    for c in range(NCORES):
        sl = slice(c * BPC, (c + 1) * BPC)
        m = {
            "proc_time": np.ascontiguousarray(proc_time[sl]),
            "machine_order": np.ascontiguousarray(machine_order[sl]),
            "next_op_idx": np.ascontiguousarray(next_op_idx[sl]),
            "finished_jobs": np.ascontiguousarray(finished_jobs[sl]),
        }
        m.update(wmap)
        in_maps.append(m)
    return in_maps


def assemble(results):
    h16 = np.concatenate([r["h_out"] for r in results], axis=0).reshape(B, N, E)
    h = h16.astype(np.float32) * np.float32(SCALE)
    je = np.concatenate([r["je_out"] for r in results], axis=0).reshape(B, J, E)
    return je, h


def run_hw(in_maps, trace=False):
    from concourse.bass_utils import run_bass_kernel_spmd

    nc = _get_nc()
    return run_bass_kernel_spmd(
        nc, in_maps, core_ids=list(range(NCORES)), trace=trace
    )


def kernel(proc_time, machine_order, next_op_idx, finished_jobs, params):
    in_maps = make_in_maps(
        proc_time, machine_order, next_op_idx, finished_jobs, params
    )
    res = run_hw(in_maps, trace=False)
    return assemble(res.results)


# revision 19
# speedup vs baseline: 1.0682x; 1.0682x over previous
"""Trainium2 Bass kernel for the L2D job-shop GNN encoder.

Problem: B=64 batches, J=50 jobs x M=20 machine-ops = N=1000 nodes, E=128,
FF=512, L=3 GNN layers.  Data-parallel over 8 NeuronCores (8 batches each).

Key algebraic restructure vs. the reference:
  - adj_prec aggregation == shift h by one token (zero at job boundaries)
  - adj_mach aggregation == P @ (P^T h) - h  with P the [N,20] one-hot of
    machine ids  ->  two tiny matmuls instead of a [1000x1000] dense matmul.
    (P S) @ Wm == P @ (S @ Wm), and the "- h @ Wm" term folds into
    (Ws - Wm) as the self-weight.
Everything runs feature-major (H = h^T, [E=128 partitions, N tokens free]);
token-contractions (machine segment-sum, final gather) use 8 TensorE
transposes per batch/layer into token-major [125,128] chunks.

All large matmuls run in float32r (single-pass replicated fp32, 4x the
fp32 rate for moving dim >= 256); producers of matmul operands write
float32r so the BIR verifier's rounding requirement is met.
"""

import numpy as np

B, J, M = 64, 50, 20
N = J * M            # 1000
E, FF, L = 128, 512, 3
NCORES = 8
BPC = B // NCORES    # 8 batches per core
CH = 8               # token chunks per batch
CP = N // CH         # 125 tokens per chunk
HALF = N // 2        # 500 (psum bank = 512 fp32)
NP = 1024            # padded token count (DMA-transpose needs 32-aligned tiles)
SCALE = 2048.0       # h is computed as h/SCALE on device (fp16 range safety)

_CACHE = {}


def _build_nc():
    import concourse.bass as bass  # noqa: F401
    import concourse.mybir as mybir
    import concourse.tile as tile
    from concourse import bacc
    from concourse.masks import make_identity

    dt = mybir.dt
    f32 = dt.float32
    f32r = dt.float32r  # noqa: F841
    f16 = dt.float16
    i32 = dt.int32
    u8 = dt.uint8
    AF = mybir.ActivationFunctionType
    OP = mybir.AluOpType

    nc = bacc.Bacc(
        "TRN2",
        target_bir_lowering=False,
        debug=False,
        enable_asserts=False,
        num_devices=NCORES,
    )

    proc = nc.dram_tensor("proc_time", [BPC, N], f32, kind="ExternalInput")
    mo = nc.dram_tensor("machine_order", [BPC, N], i32, kind="ExternalInput")
    nxt = nc.dram_tensor("next_op_idx", [BPC, J], i32, kind="ExternalInput")
    fin = nc.dram_tensor("finished_jobs", [BPC, J], u8, kind="ExternalInput")
    w_init = nc.dram_tensor("W_init", [1, E], f32, kind="ExternalInput")
    b_init = nc.dram_tensor("b_init", [E], f32, kind="ExternalInput")
    Ws_d, Wp_d, Wm_d, b_d, W1_d, b1_d, W2_d, b2_d = [], [], [], [], [], [], [], []
    for l in range(L):
        Ws_d.append(nc.dram_tensor(f"Ws{l}", [E, E], f32, kind="ExternalInput"))
        Wp_d.append(nc.dram_tensor(f"Wp{l}", [E, E], f32, kind="ExternalInput"))
        Wm_d.append(nc.dram_tensor(f"Wm{l}", [E, E], f32, kind="ExternalInput"))
        b_d.append(nc.dram_tensor(f"b{l}", [E], f32, kind="ExternalInput"))
        W1_d.append(nc.dram_tensor(f"W1{l}", [E, FF], f32, kind="ExternalInput"))
        b1_d.append(nc.dram_tensor(f"b1{l}", [FF], f32, kind="ExternalInput"))
        W2_d.append(nc.dram_tensor(f"W2{l}", [FF, E], f32, kind="ExternalInput"))
        b2_d.append(nc.dram_tensor(f"b2{l}", [E], f32, kind="ExternalInput"))
    h_out = nc.dram_tensor("h_out", [BPC, N, E], f16, kind="ExternalOutput")
    je_out = nc.dram_tensor("je_out", [BPC, J, E], f32, kind="ExternalOutput")

    with tile.TileContext(nc) as tc:
        with (
            tc.tile_pool(name="const", bufs=1) as const,
            tc.tile_pool(name="wpool", bufs=1) as wpool,
            tc.tile_pool(name="inp", bufs=4) as inp,
            tc.tile_pool(name="dramp", bufs=3, space="DRAM") as dramp,
            tc.tile_pool(name="persist", bufs=BPC + 1) as persist,
            tc.tile_pool(name="hpool", bufs=BPC + 4) as hpool,
            tc.tile_pool(name="apool", bufs=4) as apool,
            tc.tile_pool(name="htokp", bufs=4) as htokp,
            tc.tile_pool(name="msgp", bufs=4) as msgp,
            tc.tile_pool(name="tpool", bufs=5) as tpool,
            tc.tile_pool(name="smallsb", bufs=5) as smallsb,
            tc.tile_pool(name="small_ps", bufs=2, space="PSUM") as small_ps,
            tc.tile_pool(name="mt_ps", bufs=4, space="PSUM") as mt_ps,
            tc.tile_pool(name="hn_ps", bufs=2, space="PSUM") as hn_ps,
        ):
            # ---------------- constants ----------------
            ident = const.tile([128, 128], f32)
            make_identity(nc, ident[:])
            ones = const.tile([1, 128], f32)
            nc.gpsimd.memset(ones[:], 1.0)
            iota20i = const.tile([20, 1], i32)
            nc.gpsimd.iota(iota20i[:], pattern=[[0, 1]], base=0, channel_multiplier=1)
            iota20f = const.tile([20, 1], f32)
            nc.vector.tensor_copy(iota20f[:], iota20i[:])
            # token id of (partition p, chunk c) after DMA transpose = c*128 + p
            tokidi = const.tile([E, CH], i32)
            nc.gpsimd.iota(tokidi[:], pattern=[[E, CH]], base=0,
                           channel_multiplier=1)
            tokidf = const.tile([E, CH], f32)
            nc.vector.tensor_copy(tokidf[:], tokidi[:])
            iotami = const.tile([E, CH * M], i32)
            nc.gpsimd.iota(
                iotami[:], pattern=[[0, CH], [1, M]], base=0, channel_multiplier=0
            )
            iotamf = const.tile([E, CH * M], f32)
            nc.vector.tensor_copy(iotamf[:], iotami[:])
            iotaji = const.tile([1, J], i32)
            nc.gpsimd.iota(iotaji[:], pattern=[[M, J]], base=0, channel_multiplier=0)
            iotajf = const.tile([1, J], f32)
            nc.vector.tensor_copy(iotajf[:], iotaji[:])
            zero_f = const.tile([E, 1], f32)
            nc.gpsimd.memset(zero_f[:], 0.0)
            zero_h = const.tile([E, 1], f16)
            nc.vector.tensor_copy(zero_h[:], zero_f[:])
            ident_h = const.tile([128, 128], f16)
            nc.vector.tensor_copy(ident_h[:], ident[:])

            # ---------------- weights ----------------
            winit_sb = wpool.tile([E, 1], f32)
            nc.sync.dma_start(winit_sb[:], w_init.rearrange("o p -> p o"))
            binit_sb = wpool.tile([E, 1], f32)
            nc.sync.dma_start(binit_sb[:], b_init.rearrange("(p o) -> p o", o=1))
            wsm_sb, wp_sb, wm_sb, b_sb, w1_sb, b1_sb, w2_sb, b2_sb = (
                [], [], [], [], [], [], [], []
            )
            for l in range(L):
                ws_t = wpool.tile([E, E], f32, name=f"ws{l}")
                nc.sync.dma_start(ws_t[:], Ws_d[l][:])
                wpf_t = wpool.tile([E, E], f32, name=f"wpf{l}")
                nc.sync.dma_start(wpf_t[:], Wp_d[l][:])
                wmf_t = wpool.tile([E, E], f32, name=f"wmf{l}")
                nc.sync.dma_start(wmf_t[:], Wm_d[l][:])
                # f32r (rounded) weight copies for the PE
                wsm_t = wpool.tile([E, E], f16, name=f"wsm{l}")
                nc.vector.tensor_sub(wsm_t[:], ws_t[:], wmf_t[:])
                wp_t = wpool.tile([E, E], f16, name=f"wp{l}")
                nc.vector.tensor_copy(wp_t[:], wpf_t[:])
                wm_t = wpool.tile([E, E], f16, name=f"wm{l}")
                nc.vector.tensor_copy(wm_t[:], wmf_t[:])
                b_t = wpool.tile([E, 1], f32, name=f"b{l}")
                nc.sync.dma_start(b_t[:], b_d[l].rearrange("(p o) -> p o", o=1))
                w1f_t = wpool.tile([E, FF], f32, name=f"w1f{l}")
                nc.sync.dma_start(w1f_t[:], W1_d[l][:])
                w1_t = wpool.tile([E, FF], f16, name=f"w1{l}")
                nc.vector.tensor_copy(w1_t[:], w1f_t[:])
                b1_t = wpool.tile([E, FF // E], f32, name=f"b1{l}")
                nc.sync.dma_start(b1_t[:], b1_d[l].rearrange("(f p) -> p f", p=E))
                w2f_t = wpool.tile([E, FF // E, E], f32, name=f"w2f{l}")
                nc.sync.dma_start(w2f_t[:], W2_d[l].rearrange("(f p) e -> p f e", p=E))
                w2_t = wpool.tile([E, FF // E, E], f16, name=f"w2{l}")
                nc.vector.tensor_copy(w2_t[:], w2f_t[:])
                b2_t = wpool.tile([E, 1], f32, name=f"b2{l}")
                nc.sync.dma_start(b2_t[:], b2_d[l].rearrange("(p o) -> p o", o=1))
                wsm_sb.append(wsm_t)
                wp_sb.append(wp_t)
                wm_sb.append(wm_t)
                b_sb.append(b_t)
                w1_sb.append(w1_t)
                b1_sb.append(b1_t)
                w2_sb.append(w2_t)
                b2_sb.append(b2_t)

            nf = FF // E  # 4

            # ---------- prep thunks (interleaved into early denses) ----------
            pts, p_alls, g_alls, h_curs = (
                [None] * BPC, [None] * BPC, [None] * BPC, [None] * BPC
            )

            def emit_prep(b):
                st = {}
                thunks = []

                def p_dma():
                    st["mo_bc"] = inp.tile([20, N], i32, name="mo_bc")
                    nc.sync.dma_start(st["mo_bc"][:], mo[b].partition_broadcast(20))
                    st["motok_i"] = inp.tile([E, CH], i32, name="motok_i")
                    nc.gpsimd.memset(st["motok_i"][:], -1)
                    nfull = N // E  # 7 full chunks of 128 tokens
                    nc.sync.dma_start(
                        st["motok_i"][:, 0:nfull],
                        mo[b][0:nfull * E].rearrange("(c p) -> p c", p=E),
                    )
                    nc.sync.dma_start(
                        st["motok_i"][0:N - nfull * E, nfull:nfull + 1],
                        mo[b][nfull * E:N].rearrange("(p o) -> p o", o=1),
                    )
                    st["nxt_i"] = inp.tile([1, J], i32, name="nxt_i")
                    nc.sync.dma_start(st["nxt_i"][:], nxt[b][None, :])
                    st["fin_u"] = inp.tile([1, J], u8, name="fin_u")
                    nc.sync.dma_start(st["fin_u"][:], fin[b][None, :])
                    st["dur_bc"] = inp.tile([E, N], f32, name="dur_bc")
                    nc.sync.dma_start(
                        st["dur_bc"][:], proc[b].partition_broadcast(E)
                    )

                def p_onehot():
                    motok_f = inp.tile([E, CH], f32, name="motok_f")
                    nc.vector.tensor_copy(motok_f[:], st["motok_i"][:])
                    p_all = persist.tile([E, CH, M], f16, name="p_all")
                    nc.vector.tensor_tensor(
                        p_all[:],
                        motok_f[:][:, :, None].broadcast_to([E, CH, M]),
                        iotamf[:].rearrange("p (c m) -> p c m", m=M),
                        op=OP.is_equal,
                    )
                    p_alls[b] = p_all

                def p_pt():
                    pt = persist.tile([20, N], f16, name="pt")
                    nc.vector.tensor_scalar(
                        pt[:], st["mo_bc"][:], iota20f[:, 0:1], None,
                        op0=OP.is_equal,
                    )
                    pts[b] = pt

                def p_h0():
                    h_cur = hpool.tile([E, NP], f16, name="h0", tag="h")
                    nc.scalar.activation(
                        h_cur[:, 0:N], st["dur_bc"][:], AF.Identity,
                        bias=binit_sb[:, 0:1], scale=winit_sb[:, 0:1],
                    )
                    nc.vector.tensor_copy(
                        h_cur[:, N:NP], zero_h[:, 0:1].broadcast_to([E, NP - N])
                    )
                    h_curs[b] = h_cur

                def p_flat():
                    nxt_f = inp.tile([1, J], f32, name="nxt_f")
                    nc.vector.tensor_copy(nxt_f[:], st["nxt_i"][:])
                    fin_f = inp.tile([1, J], f32, name="fin_f")
                    nc.vector.tensor_copy(fin_f[:], st["fin_u"][:])
                    flat_f = inp.tile([1, J], f32, name="flat_f")
                    nc.vector.tensor_scalar(
                        flat_f[:], nxt_f[:], -1.0, 19.0, op0=OP.mult, op1=OP.add
                    )
                    nc.vector.tensor_mul(flat_f[:], flat_f[:], fin_f[:])
                    nc.vector.tensor_add(flat_f[:], flat_f[:], nxt_f[:])
                    nc.vector.tensor_add(flat_f[:], flat_f[:], iotajf[:])
                    flat_d = dramp.tile([1, J], f32, name="flat_d")
                    nc.sync.dma_start(flat_d[:], flat_f[:])
                    st["flat_bc"] = inp.tile([E, J], f32, name="flat_bc")
                    nc.sync.dma_start(
                        st["flat_bc"][:], flat_d[0].partition_broadcast(E)
                    )

                def p_g():
                    g_all = persist.tile([E, CH, J], f16, name="g_all")
                    nc.vector.tensor_tensor(
                        g_all[:],
                        st["flat_bc"][:][:, None, :].broadcast_to([E, CH, J]),
                        tokidf[:][:, :, None].broadcast_to([E, CH, J]),
                        op=OP.is_equal,
                    )
                    g_alls[b] = g_all

                return [p_dma, p_onehot, p_pt, p_h0, p_flat, p_g]

            # ---------- software-pipelined layers ----------
            # "stall" = shift + transposes + segment-sum S + U for one (b, l):
            # sparse PE work that would starve the array (HAM re-throttle) if
            # emitted as a block, so it is interleaved into the previous
            # batch's dense msg/FFN matmul stream via thunks.
            def emit_stall(b, l):
                st = {}
                thunks = []

                def sh():
                    h_in = h_curs[b]
                    agg = apool.tile([E, N], f16, name="agg", tag="agg")
                    agg3 = agg[:].rearrange("p (j s) -> p j s", s=M)
                    h3 = h_in[:, 0:N].rearrange("p (j s) -> p j s", s=M)
                    nc.vector.tensor_copy(agg3[:, :, 0:M - 1], h3[:, :, 1:M])
                    nc.vector.tensor_copy(
                        agg3[:, :, M - 1], zero_h[:, 0:1].broadcast_to([E, J])
                    )
                    st["agg"] = agg
                    st["htok"] = htokp.tile([E, CH, E], f16, name="htok",
                                            tag="htok")

                thunks.append(sh)

                def tdma():
                    # token-major copy of h via DMA xbar transpose:
                    # htok[p, c, e] = h[token c*128+p, e]
                    nc.scalar.dma_start(
                        st["htok"][:], h_curs[b][:], transpose=True
                    )

                thunks.append(tdma)

                def mk_s(c0):
                    def smm():
                        if c0 == 0:
                            st["s_ps"] = small_ps.tile([M, E], f32, name="s_ps",
                                                       tag="sp")
                        for c in range(c0, c0 + 4):
                            nc.tensor.matmul(
                                st["s_ps"][:],
                                p_alls[b][:, c, :],
                                st["htok"][:, c, :],
                                start=(c == 0),
                                stop=(c == CH - 1),
                            )
                    return smm

                thunks.append(mk_s(0))
                thunks.append(mk_s(4))

                def strans():
                    s2 = smallsb.tile([M, E], f16, name="s2")
                    nc.vector.tensor_copy(s2[:], st["s_ps"][:])
                    st_ps = small_ps.tile([E, M], f16, name="st_ps", tag="sp")
                    nc.tensor.transpose(st_ps[:], s2[:], ident_h[:M, :M])
                    st["s_sb"] = smallsb.tile([E, M], f16, name="s_sb")
                    nc.vector.tensor_copy(st["s_sb"][:], st_ps[:])

                thunks.append(strans)

                def ufin():
                    u_ps = small_ps.tile([M, E], f32, name="u_ps", tag="sp")
                    nc.tensor.matmul(u_ps[:], st["s_sb"][:], wm_sb[l][:])
                    u_sb = smallsb.tile([M, E], f16, name="u_sb")
                    nc.vector.tensor_copy(u_sb[:], u_ps[:])
                    st["u_sb"] = u_sb

                thunks.append(ufin)
                return thunks, st

            def emit_out(b):
                st = {}
                thunks = []

                def start():
                    st["htok_o"] = htokp.tile([E, CH, E], f16, name="htok_o",
                                              tag="htok")
                    nc.scalar.dma_start(
                        st["htok_o"][:], h_curs[b][:], transpose=True
                    )

                thunks.append(start)

                def mk_g(c0):
                    def gmm():
                        if c0 == 0:
                            st["je_ps"] = small_ps.tile([E, J], f32,
                                                        name="je_ps", tag="sp")
                        for c in range(c0, c0 + 4):
                            nc.tensor.matmul(
                                st["je_ps"][:],
                                st["htok_o"][:, c, :],
                                g_alls[b][:, c, :],
                                start=(c == 0),
                                stop=(c == CH - 1),
                            )
                    return gmm

                thunks.append(mk_g(0))
                thunks.append(mk_g(4))

                def fin():
                    je_sb = smallsb.tile([E, J], f32, name="je_sb")
                    nc.vector.tensor_scalar_mul(je_sb[:], st["je_ps"][:], SCALE)
                    jet_ps = small_ps.tile([J, E], f32, name="jet_ps", tag="sp")
                    nc.tensor.transpose(jet_ps[:], je_sb[:], ident[:])
                    jet_sb = smallsb.tile([J, E], f32, name="jet_sb")
                    nc.scalar.copy(jet_sb[:], jet_ps[:])
                    nc.sync.dma_start(je_out[b], jet_sb[:])
                    nfull = N // E
                    nc.sync.dma_start(
                        h_out[b][0:nfull * E].rearrange("(c p) e -> p c e", p=E),
                        st["htok_o"][:, 0:nfull, :],
                    )
                    nc.sync.dma_start(
                        h_out[b][nfull * E:N],
                        st["htok_o"][0:N - nfull * E, nfull, :],
                    )

                thunks.append(fin)
                return thunks

            def dense(b, l, st, pend):
                def fill(k):
                    for _ in range(k):
                        if pend:
                            pend.pop(0)()

                h_in = h_curs[b]
                agg = st["agg"]
                fill(2)
                msgs = []
                for hf in range(2):
                    sl = slice(hf * HALF, (hf + 1) * HALF)
                    m_ps = mt_ps.tile([E, HALF], f32, name="m_ps", tag="mt")
                    nc.tensor.matmul(
                        m_ps[:], wsm_sb[l][:], h_in[:, sl], start=True, stop=False
                    )
                    fill(1)
                    nc.tensor.matmul(
                        m_ps[:], wp_sb[l][:], agg[:, sl], start=False, stop=False
                    )
                    fill(1)
                    nc.tensor.matmul(
                        m_ps[:], st["u_sb"][:], pts[b][:, sl],
                        start=False, stop=True,
                    )
                    msg_t = msgp.tile([E, HALF], f16, name="msg_t")
                    nc.scalar.activation(
                        msg_t[:], m_ps[:], AF.Relu, bias=b_sb[l][:, 0:1]
                    )
                    msgs.append(msg_t)
                    fill(1)
                hn_pss = [
                    hn_ps.tile([E, HALF], f32, name="hn_ps0", tag="hn"),
                    hn_ps.tile([E, HALF], f32, name="hn_ps1", tag="hn"),
                ]
                t_sbs = {}
                for f in range(nf):
                    for hf in range(2):
                        tt_ps = mt_ps.tile([E, HALF], f32, name="tt_ps", tag="mt")
                        nc.tensor.matmul(
                            tt_ps[:], w1_sb[l][:, f * E:(f + 1) * E], msgs[hf][:]
                        )
                        t_sb = tpool.tile([E, HALF], f16, name="t_sb")
                        if f % 2 == 0:
                            nc.vector.tensor_scalar(
                                t_sb[:], tt_ps[:], b1_sb[l][:, f:f + 1], 0.0,
                                op0=OP.add, op1=OP.max,
                            )
                        else:
                            nc.scalar.activation(
                                t_sb[:], tt_ps[:], AF.Relu,
                                bias=b1_sb[l][:, f:f + 1],
                            )
                        t_sbs[(f, hf)] = t_sb
                    fill(1)
                    if f >= 1:
                        for hf in range(2):
                            nc.tensor.matmul(
                                hn_pss[hf][:], w2_sb[l][:, f - 1, :],
                                t_sbs.pop((f - 1, hf))[:],
                                start=(f - 1 == 0), stop=False,
                            )
                    fill(1)
                for hf in range(2):
                    nc.tensor.matmul(
                        hn_pss[hf][:], w2_sb[l][:, nf - 1, :],
                        t_sbs.pop((nf - 1, hf))[:],
                        start=False, stop=True,
                    )
                h_nxt = hpool.tile([E, NP], f16, name=f"h{l + 1}", tag="h")
                for hf in range(2):
                    sl = slice(hf * HALF, (hf + 1) * HALF)
                    nc.vector.scalar_tensor_tensor(
                        h_nxt[:, sl], hn_pss[hf][:], b2_sb[l][:, 0:1],
                        msgs[hf][:], op0=OP.add, op1=OP.add,
                    )
                nc.vector.tensor_copy(
                    h_nxt[:, N:NP], zero_h[:, 0:1].broadcast_to([E, NP - N])
                )
                h_curs[b] = h_nxt
                fill(len(pend))

            stall_sts = {}
            for bb in (0, 1, 2):
                for t in emit_prep(bb):
                    t()
            for bb in (0, 1):
                thunks, st = emit_stall(bb, 0)
                for t in thunks:
                    t()
                stall_sts[(bb, 0)] = st
            for l in range(L):
                for b in range(BPC):
                    st = stall_sts.pop((b, l))
                    pend = []
                    if l == 0 and b + 3 < BPC:
                        pend = emit_prep(b + 3)
                    tb = b + 2
                    if tb < BPC:
                        nxt_bl = (tb, l)
                    elif l + 1 < L:
                        nxt_bl = (tb - BPC, l + 1)
                    else:
                        nxt_bl = None
                    if nxt_bl is not None:
                        spend, pst = emit_stall(*nxt_bl)
                        pend = pend + spend
                        stall_sts[nxt_bl] = pst
                    if l == L - 1 and b >= 2:
                        pend = pend + emit_out(b - 2)
                    dense(b, l, st, pend)
            for bb in (BPC - 2, BPC - 1):
                for t in emit_out(bb):
                    t()

    nc.compile()
    return nc


def _get_nc():
    if "nc" not in _CACHE:
        _CACHE["nc"] = _build_nc()
    return _CACHE["nc"]


def make_in_maps(proc_time, machine_order, next_op_idx, finished_jobs, params):
    proc_time = np.asarray(proc_time, dtype=np.float32).reshape(B, N)
    machine_order = np.asarray(machine_order, dtype=np.int32).reshape(B, N)
    next_op_idx = np.asarray(next_op_idx, dtype=np.int32).reshape(B, J)
    finished_jobs = np.asarray(finished_jobs).astype(np.uint8).reshape(B, J)
    inv = np.float32(1.0 / SCALE)
    wmap = {
        "W_init": np.ascontiguousarray(np.asarray(params["W_init"], np.float32) * inv),
        "b_init": np.ascontiguousarray(np.asarray(params["b_init"], np.float32) * inv),
    }
    for l, lp in enumerate(params["layers"]):
        wmap[f"Ws{l}"] = np.ascontiguousarray(np.asarray(lp["Ws"], np.float32))
        wmap[f"Wp{l}"] = np.ascontiguousarray(np.asarray(lp["Wp"], np.float32))
        wmap[f"Wm{l}"] = np.ascontiguousarray(np.asarray(lp["Wm"], np.float32))
        wmap[f"b{l}"] = np.ascontiguousarray(np.asarray(lp["b"], np.float32) * inv)
        wmap[f"W1{l}"] = np.ascontiguousarray(np.asarray(lp["W1"], np.float32))
        wmap[f"b1{l}"] = np.ascontiguousarray(np.asarray(lp["b1"], np.float32) * inv)
        wmap[f"W2{l}"] = np.ascontiguousarray(np.asarray(lp["W2"], np.float32))
        wmap[f"b2{l}"] = np.ascontiguousarray(np.asarray(lp["b2"], np.float32) * inv)
    in_maps = []
    for c in range(NCORES):
        sl = slice(c * BPC, (c + 1) * BPC)
        m = {
            "proc_time": np.ascontiguousarray(proc_time[sl]),
            "machine_order": np.ascontiguousarray(machine_order[sl]),
            "next_op_idx": np.ascontiguousarray(next_op_idx[sl]),
            "finished_jobs": np.ascontiguousarray(finished_jobs[sl]),
        }
        m.update(wmap)
        in_maps.append(m)
    return in_maps


def assemble(results):
    h16 = np.concatenate([r["h_out"] for r in results], axis=0).reshape(B, N, E)
    h = h16.astype(np.float32) * np.float32(SCALE)
    je = np.concatenate([r["je_out"] for r in results], axis=0).reshape(B, J, E)
    return je, h


def run_hw(in_maps, trace=False):
    from concourse.bass_utils import run_bass_kernel_spmd

    nc = _get_nc()
    return run_bass_kernel_spmd(
        nc, in_maps, core_ids=list(range(NCORES)), trace=trace
    )


def kernel(proc_time, machine_order, next_op_idx, finished_jobs, params):
    in_maps = make_in_maps(
        proc_time, machine_order, next_op_idx, finished_jobs, params
    )
    res = run_hw(in_maps, trace=False)
    return assemble(res.results)
